# revision 1
# baseline (speedup 1.0000x reference)
"""Trainium2 Bass kernel for CandidateRepresentationLayer (segment span-mean).

Strategy (data-parallel over batch, per sharding hint):
  - core c owns batches [4c, 4c+4); candidates routed to cores by batch_idx.
  - span mean over word_repr[b, sid:eid) is computed as a 128x128 @ 128x512
    f32 matmul: a one-hot span matrix W (built on host from sid/eid) times a
    128-row window of word_repr held in SBUF.  Windows start at
    {0,121,242,363,484}; spans have length <= 8 so every span fits in the
    window that contains its sid.
  - candidates are bucketed by (local_batch, window), each bucket padded to a
    multiple of 128 with capacity = max over cores, so all 8 cores run one
    identical (SPMD) instruction stream.
  - after the matmul, a per-partition scalar multiply by 1/len (alternating
    vector/scalar engines) moves PSUM -> SBUF, then DMA writes the block to
    DRAM.  Host inverse-permutes rows back to the original candidate order.
"""

import numpy as np

_B, _S, _D = 32, 512, 1024
_A, _N = 4, 65536
_NCORES, _BPC = 8, 4          # cores, batches per core
_WINS = (0, 121, 242, 363, 484)
_NW = len(_WINS)
_BLK = 128                    # candidates per matmul block
_CHUNK = 8                    # blocks per W-matrix DMA chunk
_NTILES = _BPC * _NW          # word window tiles resident in SBUF

_TRACE = False                # test harness may flip this for profiling
LAST_RESULTS = None


def _build_program(nblock, block_tile):
    """Build + schedule the SPMD Bass program.

    block_tile[blk] = index of the SBUF word-window tile (lb*_NW + wj) that
    block blk contracts against; identical across cores.
    """
    import concourse.tile as tile
    from concourse import bacc, mybir

    f32 = mybir.dt.float32
    nchunk = nblock // _CHUNK

    nc = bacc.Bacc(
        "TRN2",
        target_bir_lowering=False,
        debug=False,
        enable_asserts=False,
        num_devices=_NCORES,
    )
    word_d = nc.dram_tensor("word", [_BLK, _NTILES * _D], f32, kind="ExternalInput")
    wmat_d = nc.dram_tensor(
        "wmat", [nchunk, _BLK, _CHUNK * _BLK], f32, kind="ExternalInput"
    )
    inv_d = nc.dram_tensor("invlen", [_BLK, nblock], f32, kind="ExternalInput")
    out_d = nc.dram_tensor("out", [nblock * _BLK, _D], f32, kind="ExternalOutput")

    with tile.TileContext(nc) as tc:
        with (
            tc.tile_pool(name="persist", bufs=1) as persist,
            tc.tile_pool(name="wpool", bufs=3) as wpool,
            tc.tile_pool(name="opool", bufs=4) as opool,
            tc.tile_pool(name="psum", bufs=3, space="PSUM") as pp,
        ):
            win_sb = persist.tile([_BLK, _NTILES * _D], f32)
            nc.sync.dma_start(win_sb[:], word_d[:])
            inv_sb = persist.tile([_BLK, nblock], f32)
            nc.sync.dma_start(inv_sb[:], inv_d[:])

            for ch in range(nchunk):
                wt = wpool.tile([_BLK, _CHUNK * _BLK], f32)
                nc.sync.dma_start(wt[:], wmat_d[ch])
                for j in range(_CHUNK):
                    blk = ch * _CHUNK + j
                    ti = block_tile[blk]
                    ps = pp.tile([_BLK, _D], f32)
                    lhsT = wt[:, j * _BLK : (j + 1) * _BLK]
                    for h in range(2):
                        nc.tensor.matmul(
                            ps[:, h * 512 : (h + 1) * 512],
                            lhsT,
                            win_sb[:, ti * _D + h * 512 : ti * _D + (h + 1) * 512],
                            start=True,
                            stop=True,
                        )
                    ob = opool.tile([_BLK, _D], f32)
                    sc = inv_sb[:, blk : blk + 1]
                    if blk % 2 == 0:
                        nc.vector.tensor_scalar_mul(ob[:], ps[:], sc)
                    else:
                        nc.scalar.mul(ob[:], ps[:], sc)
                    nc.sync.dma_start(out_d[blk * _BLK : (blk + 1) * _BLK, :], ob[:])
    nc.compile()
    return nc


def kernel(word_repr, candidates_idx, anchor_loc, anchor_cls):
    global LAST_RESULTS
    word_repr = np.asarray(word_repr, dtype=np.float32)
    candidates_idx = np.asarray(candidates_idx)
    anchor_loc = np.asarray(anchor_loc)
    anchor_cls = np.asarray(anchor_cls)

    b = candidates_idx[:, 0].astype(np.int64)
    w = candidates_idx[:, 1].astype(np.int64)
    a = candidates_idx[:, 2].astype(np.int64)
    loc = anchor_loc[b, w, a]  # [N, 2]
    sid = loc[:, 0].astype(np.int64)
    eid = loc[:, 1].astype(np.int64)
    ln = eid - sid
    valid = ln > 0

    # small outputs (pure index gathers)
    label = np.where(valid, anchor_cls[b, w, a], -1).astype(anchor_cls.dtype)
    counts = np.bincount(b[valid], minlength=_B).astype(np.int32)
    loc_out = np.where(valid[:, None], loc, 0).astype(anchor_loc.dtype)

    # --- shard candidates: (core, local batch, window) buckets ---
    core = b // _BPC
    lb = b % _BPC
    wins = np.asarray(_WINS, dtype=np.int64)
    wj = np.searchsorted(wins, sid, side="right") - 1
    bucket = lb * _NW + wj
    nbuck = _BPC * _NW

    cnt = np.zeros((_NCORES, nbuck), np.int64)
    np.add.at(cnt, (core, bucket), 1)
    cap_blocks = (cnt.max(axis=0) + _BLK - 1) // _BLK  # blocks per bucket
    nblock_data = int(cap_blocks.sum())
    nchunk = (nblock_data + _CHUNK - 1) // _CHUNK
    nblock = nchunk * _CHUNK
    nslot = nblock * _BLK

    bstart = np.zeros(nbuck, np.int64)  # bucket start slot
    bstart[1:] = np.cumsum(cap_blocks * _BLK)[:-1]

    block_tile = np.zeros(nblock, np.int64)
    for bu in range(nbuck):
        s0 = int(bstart[bu]) // _BLK
        block_tile[s0 : s0 + int(cap_blocks[bu])] = bu

    # per-core slot assignment
    keys = core * nbuck + bucket
    order = np.argsort(keys, kind="stable")
    sorted_keys = keys[order]
    slot_orig = np.full((_NCORES, nslot), -1, np.int64)
    for c in range(_NCORES):
        lo_i = np.searchsorted(sorted_keys, c * nbuck)
        for bu in range(nbuck):
            hi_i = np.searchsorted(sorted_keys, c * nbuck + bu + 1)
            idxs = order[lo_i:hi_i]
            slot_orig[c, bstart[bu] : bstart[bu] + len(idxs)] = idxs
            lo_i = hi_i

    # --- per-core device inputs ---
    ks = np.arange(_BLK, dtype=np.int64)
    inv_all = np.where(valid, 1.0 / np.maximum(ln, 1), 0.0).astype(np.float32)
    win_start = wins[wj]  # per candidate

    in_maps = []
    for c in range(_NCORES):
        so = slot_orig[c]
        has = so >= 0
        soc = np.where(has, so, 0)
        lo = np.where(has, sid[soc] - win_start[soc], 1)
        hi = np.where(has & valid[soc], eid[soc] - win_start[soc], 0)
        wslot = (ks[None, :] >= lo[:, None]) & (ks[None, :] < hi[:, None])
        wblk = wslot.reshape(nblock, _BLK, _BLK).transpose(0, 2, 1)  # [blk, k, cand]
        wmat = (
            wblk.reshape(nchunk, _CHUNK, _BLK, _BLK)
            .transpose(0, 2, 1, 3)
            .reshape(nchunk, _BLK, _CHUNK * _BLK)
            .astype(np.float32)
        )
        invc = np.where(has, inv_all[soc], 0.0).reshape(nblock, _BLK).T
        invc = np.ascontiguousarray(invc, dtype=np.float32)

        tiles = np.zeros((_BPC, _NW, _BLK, _D), np.float32)
        wr = word_repr[c * _BPC : (c + 1) * _BPC]
        for j, s0 in enumerate(_WINS):
            n = min(_BLK, _S - s0)
            tiles[:, j, :n, :] = wr[:, s0 : s0 + n, :]
        word_in = np.ascontiguousarray(
            tiles.transpose(2, 0, 1, 3).reshape(_BLK, _NTILES * _D)
        )
        in_maps.append({"word": word_in, "wmat": wmat, "invlen": invc})

    # --- build, run, unshard ---
    nc = _build_program(nblock, [int(t) for t in block_tile])
    from concourse.bass_utils import run_bass_kernel_spmd

    res = run_bass_kernel_spmd(
        nc, in_maps, core_ids=list(range(_NCORES)), trace=_TRACE
    )
    LAST_RESULTS = res

    repr_ = np.zeros((_N, _D), np.float32)
    for c in range(_NCORES):
        oc = res.results[c]["out"]
        so = slot_orig[c]
        m = so >= 0
        repr_[so[m]] = oc[m]
    return repr_, label, counts, valid, loc_out


# revision 3
# speedup vs baseline: 1.0486x; 1.0486x over previous
"""Trainium2 Bass kernel for CandidateRepresentationLayer (segment span-mean).

Strategy (data-parallel over batch, per sharding hint):
  - core c owns batches [4c, 4c+4); candidates routed to cores by batch_idx.
  - span mean over word_repr[b, sid:eid) is computed as a 128x128 @ 128x512
    f32 matmul: a one-hot span matrix W (built on host from sid/eid) times a
    128-row window of word_repr held in SBUF.  Windows start at
    {0,121,242,363,484}; spans have length <= 8 so every span fits in the
    window that contains its sid.
  - candidates are bucketed by (local_batch, window), each bucket padded to a
    multiple of 128 with capacity = max over cores, so all 8 cores run one
    identical (SPMD) instruction stream.
  - after the matmul, a per-partition scalar multiply by 1/len (alternating
    vector/scalar engines) moves PSUM -> SBUF, then DMA writes the block to
    DRAM.  Host inverse-permutes rows back to the original candidate order.
"""

import numpy as np

_B, _S, _D = 32, 512, 1024
_A, _N = 4, 65536
_NCORES, _BPC = 8, 4          # cores, batches per core
_WINS = (0, 121, 242, 363, 484)
_NW = len(_WINS)
_BLK = 128                    # candidates per matmul block
_CHUNK = 8                    # blocks per W-matrix DMA chunk
_NTILES = _BPC * _NW          # word window tiles resident in SBUF

_TRACE = False                # test harness may flip this for profiling
LAST_RESULTS = None


def _build_program(nblock, block_tile):
    """Build + schedule the SPMD Bass program.

    block_tile[blk] = index of the SBUF word-window tile (lb*_NW + wj) that
    block blk contracts against; identical across cores.

    word windows are shipped as a bf16 hi/lo split (word = hi + lo exactly to
    ~2^-17 relative), so the span matmul runs at bf16 speed (1 cyc/row vs 4
    for f32) with two accumulating matmuls per PSUM half.
    """
    import concourse.tile as tile
    from concourse import bacc, mybir

    f32 = mybir.dt.float32
    bf16 = mybir.dt.bfloat16
    nchunk = nblock // _CHUNK

    nc = bacc.Bacc(
        "TRN2",
        target_bir_lowering=False,
        debug=False,
        enable_asserts=False,
        num_devices=_NCORES,
    )
    # [p, (tile, half, d)] — half 0 = hi, 1 = lo
    word_d = nc.dram_tensor(
        "word", [_BLK, _NTILES * 2 * _D], bf16, kind="ExternalInput"
    )
    wmat_d = nc.dram_tensor(
        "wmat", [nchunk, _BLK, _CHUNK * _BLK], bf16, kind="ExternalInput"
    )
    inv_d = nc.dram_tensor("invlen", [_BLK, nblock], f32, kind="ExternalInput")
    out_d = nc.dram_tensor("out", [nblock * _BLK, _D], f32, kind="ExternalOutput")

    with tile.TileContext(nc) as tc:
        with (
            tc.tile_pool(name="persist", bufs=1) as persist,
            tc.tile_pool(name="wpool", bufs=3) as wpool,
            tc.tile_pool(name="opool", bufs=4) as opool,
            tc.tile_pool(name="psum", bufs=3, space="PSUM") as pp,
        ):
            win_sb = persist.tile([_BLK, _NTILES * 2 * _D], bf16)
            nc.sync.dma_start(win_sb[:], word_d[:])
            inv_sb = persist.tile([_BLK, nblock], f32)
            nc.sync.dma_start(inv_sb[:], inv_d[:])

            for ch in range(nchunk):
                wt = wpool.tile([_BLK, _CHUNK * _BLK], bf16)
                nc.sync.dma_start(wt[:], wmat_d[ch])
                for j in range(_CHUNK):
                    blk = ch * _CHUNK + j
                    ti = block_tile[blk]
                    ps = pp.tile([_BLK, _D], f32)
                    lhsT = wt[:, j * _BLK : (j + 1) * _BLK]
                    base = ti * 2 * _D
                    for h in range(2):
                        for half in range(2):  # hi, lo accumulate
                            o = base + half * _D + h * 512
                            nc.tensor.matmul(
                                ps[:, h * 512 : (h + 1) * 512],
                                lhsT,
                                win_sb[:, o : o + 512],
                                start=(half == 0),
                                stop=(half == 1),
                            )
                    ob = opool.tile([_BLK, _D], f32)
                    sc = inv_sb[:, blk : blk + 1]
                    if blk % 2 == 0:
                        nc.vector.tensor_scalar_mul(ob[:], ps[:], sc)
                    else:
                        nc.scalar.mul(ob[:], ps[:], sc)
                    nc.sync.dma_start(out_d[blk * _BLK : (blk + 1) * _BLK, :], ob[:])
    nc.compile()
    return nc


def kernel(word_repr, candidates_idx, anchor_loc, anchor_cls):
    global LAST_RESULTS
    word_repr = np.asarray(word_repr, dtype=np.float32)
    candidates_idx = np.asarray(candidates_idx)
    anchor_loc = np.asarray(anchor_loc)
    anchor_cls = np.asarray(anchor_cls)

    b = candidates_idx[:, 0].astype(np.int64)
    w = candidates_idx[:, 1].astype(np.int64)
    a = candidates_idx[:, 2].astype(np.int64)
    loc = anchor_loc[b, w, a]  # [N, 2]
    sid = loc[:, 0].astype(np.int64)
    eid = loc[:, 1].astype(np.int64)
    ln = eid - sid
    valid = ln > 0

    # small outputs (pure index gathers)
    label = np.where(valid, anchor_cls[b, w, a], -1).astype(anchor_cls.dtype)
    counts = np.bincount(b[valid], minlength=_B).astype(np.int32)
    loc_out = np.where(valid[:, None], loc, 0).astype(anchor_loc.dtype)

    # --- shard candidates: (core, local batch, window) buckets ---
    core = b // _BPC
    lb = b % _BPC
    wins = np.asarray(_WINS, dtype=np.int64)
    wj = np.searchsorted(wins, sid, side="right") - 1
    bucket = lb * _NW + wj
    nbuck = _BPC * _NW

    cnt = np.zeros((_NCORES, nbuck), np.int64)
    np.add.at(cnt, (core, bucket), 1)
    cap_blocks = (cnt.max(axis=0) + _BLK - 1) // _BLK  # blocks per bucket
    nblock_data = int(cap_blocks.sum())
    nchunk = (nblock_data + _CHUNK - 1) // _CHUNK
    nblock = nchunk * _CHUNK
    nslot = nblock * _BLK

    bstart = np.zeros(nbuck, np.int64)  # bucket start slot
    bstart[1:] = np.cumsum(cap_blocks * _BLK)[:-1]

    block_tile = np.zeros(nblock, np.int64)
    for bu in range(nbuck):
        s0 = int(bstart[bu]) // _BLK
        block_tile[s0 : s0 + int(cap_blocks[bu])] = bu

    # per-core slot assignment
    keys = core * nbuck + bucket
    order = np.argsort(keys, kind="stable")
    sorted_keys = keys[order]
    slot_orig = np.full((_NCORES, nslot), -1, np.int64)
    for c in range(_NCORES):
        lo_i = np.searchsorted(sorted_keys, c * nbuck)
        for bu in range(nbuck):
            hi_i = np.searchsorted(sorted_keys, c * nbuck + bu + 1)
            idxs = order[lo_i:hi_i]
            slot_orig[c, bstart[bu] : bstart[bu] + len(idxs)] = idxs
            lo_i = hi_i

    # --- per-core device inputs ---
    import ml_dtypes

    bf16 = ml_dtypes.bfloat16
    ks = np.arange(_BLK, dtype=np.int64)
    inv_all = np.where(valid, 1.0 / np.maximum(ln, 1), 0.0).astype(np.float32)
    win_start = wins[wj]  # per candidate

    word_hi = word_repr.astype(bf16)
    word_lo = (word_repr - word_hi.astype(np.float32)).astype(bf16)

    in_maps = []
    for c in range(_NCORES):
        so = slot_orig[c]
        has = so >= 0
        soc = np.where(has, so, 0)
        lo = np.where(has, sid[soc] - win_start[soc], 1)
        hi = np.where(has & valid[soc], eid[soc] - win_start[soc], 0)
        wslot = (ks[None, :] >= lo[:, None]) & (ks[None, :] < hi[:, None])
        wblk = wslot.reshape(nblock, _BLK, _BLK).transpose(0, 2, 1)  # [blk, k, cand]
        wmat = (
            wblk.reshape(nchunk, _CHUNK, _BLK, _BLK)
            .transpose(0, 2, 1, 3)
            .reshape(nchunk, _BLK, _CHUNK * _BLK)
            .astype(bf16)
        )
        invc = np.where(has, inv_all[soc], 0.0).reshape(nblock, _BLK).T
        invc = np.ascontiguousarray(invc, dtype=np.float32)

        tiles = np.zeros((_BPC, _NW, 2, _BLK, _D), bf16)
        for half, wsrc in enumerate((word_hi, word_lo)):
            wr = wsrc[c * _BPC : (c + 1) * _BPC]
            for j, s0 in enumerate(_WINS):
                n = min(_BLK, _S - s0)
                tiles[:, j, half, :n, :] = wr[:, s0 : s0 + n, :]
        # -> [p, (lb, wj, half, d)]
        word_in = np.ascontiguousarray(
            tiles.transpose(3, 0, 1, 2, 4).reshape(_BLK, _NTILES * 2 * _D)
        )
        in_maps.append({"word": word_in, "wmat": wmat, "invlen": invc})

    # --- build, run, unshard ---
    nc = _build_program(nblock, [int(t) for t in block_tile])
    from concourse.bass_utils import run_bass_kernel_spmd

    res = run_bass_kernel_spmd(
        nc, in_maps, core_ids=list(range(_NCORES)), trace=_TRACE
    )
    LAST_RESULTS = res

    repr_ = np.zeros((_N, _D), np.float32)
    for c in range(_NCORES):
        oc = res.results[c]["out"]
        so = slot_orig[c]
        m = so >= 0
        repr_[so[m]] = oc[m]
    return repr_, label, counts, valid, loc_out


# revision 4
# speedup vs baseline: 1.2764x; 1.2173x over previous
"""Trainium2 Bass kernel for CandidateRepresentationLayer (segment span-mean).

Strategy (data-parallel over batch, per sharding hint):
  - core c owns batches [4c, 4c+4); candidates routed to cores by batch_idx.
  - span mean over word_repr[b, sid:eid) is computed as a one-hot matmul:
    a span matrix W (built on host from sid/eid) times a 128-row window of
    word_repr held in SBUF.  Windows start at {0,121,242,363,484}; spans have
    length <= 8 so every span fits in the window containing its sid.
  - word windows are shipped as a bf16 hi/lo split (word = hi + lo, exact to
    ~2^-17 relative), so the matmul runs at bf16 speed with two accumulating
    matmuls per PSUM half; the result is f32-accurate to ~1e-5.
  - candidates are bucketed by (local_batch, window); bucket capacity =
    max count over cores rounded to 32, so all 8 cores run one identical
    (SPMD) instruction stream.  Buckets are cut into matmul blocks of up to
    128 candidates (partition dim); tail blocks are narrower to cut padded
    output DMA.
  - after the matmul, a per-partition scalar multiply by 1/len (alternating
    vector/scalar engines) moves PSUM -> SBUF, then DMA writes the block.
    Host inverse-permutes rows back to candidate order and computes the tiny
    index outputs (label/counts/valid/loc) directly.
"""

import numpy as np

_B, _S, _D = 32, 512, 1024
_A, _N = 4, 65536
_NCORES, _BPC = 8, 4          # cores, batches per core
_WINS = (0, 121, 242, 363, 484)
_NW = len(_WINS)
_BLK = 128                    # max candidates per matmul block
_GRAN = 32                    # bucket capacity granularity
_CHUNK_SLOTS = 1024           # W-matrix DMA chunk size (slots)
_NTILES = _BPC * _NW          # word window tiles resident in SBUF

_TRACE = False                # test harness may flip this for profiling
LAST_RESULTS = None


def _plan(cap_slots):
    """Cut bucket capacities into blocks and W-DMA chunks.

    Returns (nslot, blocks, chunks); blocks = (slot0, m, bucket);
    chunks = (slot0, nslots, [block ids]).
    """
    blocks = []
    off = 0
    for bu, cap in enumerate(cap_slots):
        rem = int(cap)
        while rem > 0:
            m = min(_BLK, rem)
            blocks.append((off, m, bu))
            off += m
            rem -= m
    nslot = off
    chunks = []
    cur = []
    c0 = 0
    cs = 0
    for bi, (s0, m, _) in enumerate(blocks):
        if cs + m > _CHUNK_SLOTS and cur:
            chunks.append((c0, cs, cur))
            c0, cs, cur = s0, 0, []
        cur.append(bi)
        cs += m
    if cur:
        chunks.append((c0, cs, cur))
    return nslot, blocks, chunks


def _build_program(nslot, blocks, chunks):
    import concourse.tile as tile
    from concourse import bacc, mybir

    f32 = mybir.dt.float32
    bf16 = mybir.dt.bfloat16
    nblock = len(blocks)

    nc = bacc.Bacc(
        "TRN2",
        target_bir_lowering=False,
        debug=False,
        enable_asserts=False,
        num_devices=_NCORES,
    )
    # [p, (tile, half, d)] — half 0 = hi, 1 = lo
    word_d = nc.dram_tensor(
        "word", [_BLK, _NTILES * 2 * _D], bf16, kind="ExternalInput"
    )
    wmat_d = nc.dram_tensor("wmat", [_BLK, nslot], bf16, kind="ExternalInput")
    inv_d = nc.dram_tensor("invlen", [_BLK, nblock], f32, kind="ExternalInput")
    out_d = nc.dram_tensor("out", [nslot, _D], f32, kind="ExternalOutput")

    with tile.TileContext(nc) as tc:
        with (
            tc.tile_pool(name="persist", bufs=1) as persist,
            tc.tile_pool(name="wpool", bufs=3) as wpool,
            tc.tile_pool(name="opool", bufs=4) as opool,
            tc.tile_pool(name="psum", bufs=3, space="PSUM") as pp,
        ):
            win_sb = persist.tile([_BLK, _NTILES * 2 * _D], bf16)
            # split the window upload so compute starts after the first tile
            for t in range(_NTILES):
                o = t * 2 * _D
                nc.sync.dma_start(
                    win_sb[:, o : o + 2 * _D], word_d[:, o : o + 2 * _D]
                )
            inv_sb = persist.tile([_BLK, nblock], f32)
            nc.sync.dma_start(inv_sb[:], inv_d[:])

            for c0, cs, bids in chunks:
                wt = wpool.tile([_BLK, _CHUNK_SLOTS], bf16, tag="wt")
                nc.sync.dma_start(wt[:, :cs], wmat_d[:, c0 : c0 + cs])
                for bi in bids:
                    s0, m, bu = blocks[bi]
                    ps = pp.tile([_BLK, _D], f32, tag="ps")
                    lhsT = wt[:, s0 - c0 : s0 - c0 + m]
                    base = bu * 2 * _D
                    for h in range(2):
                        for half in range(2):  # hi, lo accumulate
                            o = base + half * _D + h * 512
                            nc.tensor.matmul(
                                ps[:m, h * 512 : (h + 1) * 512],
                                lhsT,
                                win_sb[:, o : o + 512],
                                start=(half == 0),
                                stop=(half == 1),
                            )
                    ob = opool.tile([_BLK, _D], f32, tag="ob")
                    sc = inv_sb[:m, bi : bi + 1]
                    if bi % 2 == 0:
                        nc.vector.tensor_scalar_mul(ob[:m], ps[:m], sc)
                    else:
                        nc.scalar.mul(ob[:m], ps[:m], sc)
                    nc.sync.dma_start(out_d[s0 : s0 + m, :], ob[:m])
    nc.compile()
    return nc


def kernel(word_repr, candidates_idx, anchor_loc, anchor_cls):
    global LAST_RESULTS
    word_repr = np.asarray(word_repr, dtype=np.float32)
    candidates_idx = np.asarray(candidates_idx)
    anchor_loc = np.asarray(anchor_loc)
    anchor_cls = np.asarray(anchor_cls)

    b = candidates_idx[:, 0].astype(np.int64)
    w = candidates_idx[:, 1].astype(np.int64)
    a = candidates_idx[:, 2].astype(np.int64)
    loc = anchor_loc[b, w, a]  # [N, 2]
    sid = loc[:, 0].astype(np.int64)
    eid = loc[:, 1].astype(np.int64)
    ln = eid - sid
    valid = ln > 0

    # small outputs (pure index gathers)
    label = np.where(valid, anchor_cls[b, w, a], -1).astype(anchor_cls.dtype)
    counts = np.bincount(b[valid], minlength=_B).astype(np.int32)
    loc_out = np.where(valid[:, None], loc, 0).astype(anchor_loc.dtype)

    # --- shard candidates: (core, local batch, window) buckets ---
    core = b // _BPC
    lb = b % _BPC
    wins = np.asarray(_WINS, dtype=np.int64)
    wj = np.searchsorted(wins, sid, side="right") - 1
    bucket = lb * _NW + wj
    nbuck = _BPC * _NW

    cnt = np.zeros((_NCORES, nbuck), np.int64)
    np.add.at(cnt, (core, bucket), 1)
    cap_slots = ((cnt.max(axis=0) + _GRAN - 1) // _GRAN) * _GRAN
    nslot, blocks, chunks = _plan(cap_slots)
    nblock = len(blocks)

    bstart = np.zeros(nbuck, np.int64)  # bucket start slot
    bstart[1:] = np.cumsum(cap_slots)[:-1]

    # per-core slot assignment
    keys = core * nbuck + bucket
    order = np.argsort(keys, kind="stable")
    sorted_keys = keys[order]
    slot_orig = np.full((_NCORES, nslot), -1, np.int64)
    for c in range(_NCORES):
        lo_i = np.searchsorted(sorted_keys, c * nbuck)
        for bu in range(nbuck):
            hi_i = np.searchsorted(sorted_keys, c * nbuck + bu + 1)
            idxs = order[lo_i:hi_i]
            slot_orig[c, bstart[bu] : bstart[bu] + len(idxs)] = idxs
            lo_i = hi_i

    # --- per-core device inputs ---
    import ml_dtypes

    bf16 = ml_dtypes.bfloat16
    ks = np.arange(_BLK, dtype=np.int64)
    inv_all = np.where(valid, 1.0 / np.maximum(ln, 1), 0.0).astype(np.float32)
    win_start = wins[wj]  # per candidate

    word_hi = word_repr.astype(bf16)
    word_lo = (word_repr - word_hi.astype(np.float32)).astype(bf16)

    in_maps = []
    for c in range(_NCORES):
        so = slot_orig[c]
        has = so >= 0
        soc = np.where(has, so, 0)
        lo = np.where(has, sid[soc] - win_start[soc], 1)
        hi = np.where(has & valid[soc], eid[soc] - win_start[soc], 0)
        # W [k, slot]
        wmat = (
            (ks[:, None] >= lo[None, :]) & (ks[:, None] < hi[None, :])
        ).astype(bf16)

        inv_slot = np.where(has, inv_all[soc], 0.0).astype(np.float32)
        invc = np.zeros((_BLK, nblock), np.float32)
        for bi, (s0, m, _) in enumerate(blocks):
            invc[:m, bi] = inv_slot[s0 : s0 + m]

        tiles = np.zeros((_BPC, _NW, 2, _BLK, _D), bf16)
        for half, wsrc in enumerate((word_hi, word_lo)):
            wr = wsrc[c * _BPC : (c + 1) * _BPC]
            for j, s0 in enumerate(_WINS):
                n = min(_BLK, _S - s0)
                tiles[:, j, half, :n, :] = wr[:, s0 : s0 + n, :]
        # -> [p, (lb, wj, half, d)]
        word_in = np.ascontiguousarray(
            tiles.transpose(3, 0, 1, 2, 4).reshape(_BLK, _NTILES * 2 * _D)
        )
        in_maps.append({"word": word_in, "wmat": wmat, "invlen": invc})

    # --- build, run, unshard ---
    nc = _build_program(nslot, blocks, chunks)
    from concourse.bass_utils import run_bass_kernel_spmd

    res = run_bass_kernel_spmd(
        nc, in_maps, core_ids=list(range(_NCORES)), trace=_TRACE
    )
    LAST_RESULTS = res

    repr_ = np.zeros((_N, _D), np.float32)
    for c in range(_NCORES):
        oc = res.results[c]["out"]
        so = slot_orig[c]
        m = so >= 0
        repr_[so[m]] = oc[m]
    return repr_, label, counts, valid, loc_out


# revision 7
# speedup vs baseline: 1.2992x; 1.0179x over previous
"""Trainium2 Bass kernel for CandidateRepresentationLayer (segment span-mean).

Strategy (data-parallel over batch, per sharding hint):
  - core c owns batches [4c, 4c+4); candidates routed to cores by batch_idx.
  - span mean over word_repr[b, sid:eid) is computed as a one-hot matmul:
    a span matrix W (built on host from sid/eid) times a 128-row window of
    word_repr held in SBUF.  Windows start at {0,121,242,363,484}; spans have
    length <= 8 so every span fits in the window containing its sid.
  - word windows are shipped as a bf16 hi/lo split (word = hi + lo, exact to
    ~2^-17 relative), so the matmul runs at bf16 speed with two accumulating
    matmuls per PSUM half; the result is f32-accurate to ~1e-5.
  - candidates are bucketed by (local_batch, window); bucket capacity =
    max count over cores rounded to 32, so all 8 cores run one identical
    (SPMD) instruction stream.  Buckets are cut into matmul blocks of up to
    128 candidates (partition dim); tail blocks are narrower to cut padded
    output DMA.
  - after the matmul, a per-partition scalar multiply by 1/len (alternating
    vector/scalar engines) moves PSUM -> SBUF, then DMA writes the block.
    Host inverse-permutes rows back to candidate order and computes the tiny
    index outputs (label/counts/valid/loc) directly.
"""

import numpy as np

_B, _S, _D = 32, 512, 1024
_A, _N = 4, 65536
_NCORES, _BPC = 8, 4          # cores, batches per core
_WINS = (0, 121, 242, 363, 484)
_NW = len(_WINS)
_BLK = 128                    # max candidates per matmul block
_GRAN = 32                    # bucket capacity granularity
_CHUNK_SLOTS = 1024           # W-matrix DMA chunk size (slots)
_NTILES = _BPC * _NW          # word window tiles resident in SBUF

_TRACE = False                # test harness may flip this for profiling
LAST_RESULTS = None


def _plan(cap_slots):
    """Cut bucket capacities into blocks and W-DMA chunks.

    Returns (nslot, blocks, chunks); blocks = (slot0, m, bucket);
    chunks = (slot0, nslots, [block ids]).
    """
    blocks = []
    off = 0
    for bu, cap in enumerate(cap_slots):
        rem = int(cap)
        while rem > 0:
            m = min(_BLK, rem)
            blocks.append((off, m, bu))
            off += m
            rem -= m
    nslot = off
    chunks = []
    cur = []
    c0 = 0
    cs = 0
    for bi, (s0, m, _) in enumerate(blocks):
        if cs + m > _CHUNK_SLOTS and cur:
            chunks.append((c0, cs, cur))
            c0, cs, cur = s0, 0, []
        cur.append(bi)
        cs += m
    if cur:
        chunks.append((c0, cs, cur))
    return nslot, blocks, chunks


def _build_program(nslot, blocks, chunks):
    import concourse.tile as tile
    from concourse import bacc, mybir

    f32 = mybir.dt.float32
    bf16 = mybir.dt.bfloat16
    nblock = len(blocks)

    nc = bacc.Bacc(
        "TRN2",
        target_bir_lowering=False,
        debug=False,
        enable_asserts=False,
        num_devices=_NCORES,
    )
    # [p, (tile, half, d)] — half 0 = hi, 1 = lo
    word_d = nc.dram_tensor(
        "word", [_BLK, _NTILES * 2 * _D], bf16, kind="ExternalInput"
    )
    wmat_d = nc.dram_tensor("wmat", [_BLK, nslot], bf16, kind="ExternalInput")
    inv_d = nc.dram_tensor("invlen", [_BLK, nblock], f32, kind="ExternalInput")
    out_d = nc.dram_tensor("out", [nslot, _D], f32, kind="ExternalOutput")

    with tile.TileContext(nc) as tc:
        with (
            tc.tile_pool(name="persist", bufs=1) as persist,
            tc.tile_pool(name="wpool", bufs=3) as wpool,
            tc.tile_pool(name="opool", bufs=4) as opool,
            tc.tile_pool(name="psum", bufs=3, space="PSUM") as pp,
            tc.tile_pool(name="pwarm", bufs=1, space="PSUM") as pw,
        ):
            win_sb = persist.tile([_BLK, _NTILES * 2 * _D], bf16)
            inv_sb = persist.tile([_BLK, nblock], f32)
            nc.sync.dma_start(inv_sb[:], inv_d[:])

            wts = {}

            def load_chunk(i):
                c0, cs, _ = chunks[i]
                wt = wpool.tile([_BLK, _CHUNK_SLOTS], bf16, tag="wt")
                nc.sync.dma_start(wt[:, :cs], wmat_d[:, c0 : c0 + cs])
                wts[i] = wt

            # Split the window upload so compute starts after the first tile;
            # prefetch the first W chunks between windows.  After each window
            # lands, run a throwaway matmul on it to keep the PE ticking
            # through the load phase (HAM stays un-throttled).
            warm = pw.tile([_BLK, 512], f32, tag="warm")
            load_chunk(0)
            for t in range(_NTILES):
                o = t * 2 * _D
                nc.sync.dma_start(
                    win_sb[:, o : o + 2 * _D], word_d[:, o : o + 2 * _D]
                )
                if t == 0:
                    load_chunk(1)
                nc.tensor.matmul(
                    warm[:],
                    win_sb[:, o : o + _BLK],
                    win_sb[:, o : o + 512],
                    start=True,
                    stop=True,
                )

            for ci, (c0, cs, bids) in enumerate(chunks):
                if ci not in wts:
                    load_chunk(ci)
                wt = wts.pop(ci)
                for bi in bids:
                    s0, m, bu = blocks[bi]
                    ps = pp.tile([_BLK, _D], f32, tag="ps")
                    lhsT = wt[:, s0 - c0 : s0 - c0 + m]
                    base = bu * 2 * _D
                    for h in range(2):
                        for half in range(2):  # hi, lo accumulate
                            o = base + half * _D + h * 512
                            nc.tensor.matmul(
                                ps[:m, h * 512 : (h + 1) * 512],
                                lhsT,
                                win_sb[:, o : o + 512],
                                start=(half == 0),
                                stop=(half == 1),
                            )
                    ob = opool.tile([_BLK, _D], f32, tag="ob")
                    sc = inv_sb[:m, bi : bi + 1]
                    if bi % 2 == 0:
                        nc.vector.tensor_scalar_mul(ob[:m], ps[:m], sc)
                    else:
                        nc.scalar.mul(ob[:m], ps[:m], sc)
                    nc.sync.dma_start(out_d[s0 : s0 + m, :], ob[:m])
    nc.compile()
    return nc


def kernel(word_repr, candidates_idx, anchor_loc, anchor_cls):
    global LAST_RESULTS
    word_repr = np.asarray(word_repr, dtype=np.float32)
    candidates_idx = np.asarray(candidates_idx)
    anchor_loc = np.asarray(anchor_loc)
    anchor_cls = np.asarray(anchor_cls)

    b = candidates_idx[:, 0].astype(np.int64)
    w = candidates_idx[:, 1].astype(np.int64)
    a = candidates_idx[:, 2].astype(np.int64)
    loc = anchor_loc[b, w, a]  # [N, 2]
    sid = loc[:, 0].astype(np.int64)
    eid = loc[:, 1].astype(np.int64)
    ln = eid - sid
    valid = ln > 0

    # small outputs (pure index gathers)
    label = np.where(valid, anchor_cls[b, w, a], -1).astype(anchor_cls.dtype)
    counts = np.bincount(b[valid], minlength=_B).astype(np.int32)
    loc_out = np.where(valid[:, None], loc, 0).astype(anchor_loc.dtype)

    # --- shard candidates: (core, local batch, window) buckets ---
    core = b // _BPC
    lb = b % _BPC
    wins = np.asarray(_WINS, dtype=np.int64)
    wj = np.searchsorted(wins, sid, side="right") - 1
    bucket = lb * _NW + wj
    nbuck = _BPC * _NW

    cnt = np.zeros((_NCORES, nbuck), np.int64)
    np.add.at(cnt, (core, bucket), 1)
    cap_slots = ((cnt.max(axis=0) + _GRAN - 1) // _GRAN) * _GRAN
    nslot, blocks, chunks = _plan(cap_slots)
    nblock = len(blocks)

    bstart = np.zeros(nbuck, np.int64)  # bucket start slot
    bstart[1:] = np.cumsum(cap_slots)[:-1]

    # per-core slot assignment
    keys = core * nbuck + bucket
    order = np.argsort(keys, kind="stable")
    sorted_keys = keys[order]
    slot_orig = np.full((_NCORES, nslot), -1, np.int64)
    for c in range(_NCORES):
        lo_i = np.searchsorted(sorted_keys, c * nbuck)
        for bu in range(nbuck):
            hi_i = np.searchsorted(sorted_keys, c * nbuck + bu + 1)
            idxs = order[lo_i:hi_i]
            slot_orig[c, bstart[bu] : bstart[bu] + len(idxs)] = idxs
            lo_i = hi_i

    # --- per-core device inputs ---
    import ml_dtypes

    bf16 = ml_dtypes.bfloat16
    ks = np.arange(_BLK, dtype=np.int64)
    inv_all = np.where(valid, 1.0 / np.maximum(ln, 1), 0.0).astype(np.float32)
    win_start = wins[wj]  # per candidate

    word_hi = word_repr.astype(bf16)
    word_lo = (word_repr - word_hi.astype(np.float32)).astype(bf16)

    in_maps = []
    for c in range(_NCORES):
        so = slot_orig[c]
        has = so >= 0
        soc = np.where(has, so, 0)
        lo = np.where(has, sid[soc] - win_start[soc], 1)
        hi = np.where(has & valid[soc], eid[soc] - win_start[soc], 0)
        # W [k, slot]
        wmat = (
            (ks[:, None] >= lo[None, :]) & (ks[:, None] < hi[None, :])
        ).astype(bf16)

        inv_slot = np.where(has, inv_all[soc], 0.0).astype(np.float32)
        invc = np.zeros((_BLK, nblock), np.float32)
        for bi, (s0, m, _) in enumerate(blocks):
            invc[:m, bi] = inv_slot[s0 : s0 + m]

        tiles = np.zeros((_BPC, _NW, 2, _BLK, _D), bf16)
        for half, wsrc in enumerate((word_hi, word_lo)):
            wr = wsrc[c * _BPC : (c + 1) * _BPC]
            for j, s0 in enumerate(_WINS):
                n = min(_BLK, _S - s0)
                tiles[:, j, half, :n, :] = wr[:, s0 : s0 + n, :]
        # -> [p, (lb, wj, half, d)]
        word_in = np.ascontiguousarray(
            tiles.transpose(3, 0, 1, 2, 4).reshape(_BLK, _NTILES * 2 * _D)
        )
        in_maps.append({"word": word_in, "wmat": wmat, "invlen": invc})

    # --- build, run, unshard ---
    nc = _build_program(nslot, blocks, chunks)
    from concourse.bass_utils import run_bass_kernel_spmd

    res = run_bass_kernel_spmd(
        nc, in_maps, core_ids=list(range(_NCORES)), trace=_TRACE
    )
    LAST_RESULTS = res

    repr_ = np.zeros((_N, _D), np.float32)
    for c in range(_NCORES):
        oc = res.results[c]["out"]
        so = slot_orig[c]
        m = so >= 0
        repr_[so[m]] = oc[m]
    return repr_, label, counts, valid, loc_out
